# revision 8
# baseline (speedup 1.0000x reference)
"""AvU loss (accuracy-vs-uncertainty) Trainium2 kernel.

Strategy (data parallel over 8 NeuronCores):
  Each sample contributes w = q*r to the denominator and w*[a==u] to the
  numerator, where
     q = c if accurate else (1-c),        c = probs[:,1]
     r = (1-t) if certain else t,         t = tanh(unc)
     a = [label == argmax(probs)],        u = [unc <= unc_th]
  With sign encodings S_a = 2a-1, S_u = 2u-1 (both +-1):
     WS2 := (S_a + c2) * (u01 - t)  where c2 = 2c-1, u01 = [unc<=th]
          = 2 * w * S_a * S_u
  so   sum(w)        = sum(|WS2|) / 2
       sum(w*[a==u]) = (sum(|WS2|) + sum(WS2)) / 4
  Each core computes per-partition partial sums of WS2 and |WS2|; the host
  combines 8 * 128 * T partials in float64 and finishes the log.
"""

import numpy as np

_N = 16777216
_NCORES = 8
_P = 128
_F = 2048
_NC = _N // _NCORES
_T = _NC // (_P * _F)  # 8 tiles per core

_built = {}


def _build(unc_th: float, lab_mode: str, T=_T, F=_F):
    import concourse.bacc as bacc
    import concourse.mybir as mybir
    import concourse.tile as tile

    f32 = mybir.dt.float32
    bf16 = mybir.dt.bfloat16
    i32 = mybir.dt.int32
    Alu = mybir.AluOpType
    Act = mybir.ActivationFunctionType

    nc = bacc.Bacc("TRN2")
    labw = 2 if lab_mode == "i64" else 1
    probs = nc.dram_tensor("probs", [T, _P, 2 * F], f32, kind="ExternalInput")
    labs = nc.dram_tensor("labs", [T, _P, labw * F], i32, kind="ExternalInput")
    unc = nc.dram_tensor("unc", [T, _P, F], f32, kind="ExternalInput")
    out = nc.dram_tensor("out", [_P, 2 * T], f32, kind="ExternalOutput")

    with tile.TileContext(nc) as tc:
        with (
            tc.tile_pool(name="io", bufs=2) as io,
            tc.tile_pool(name="mid", bufs=2) as mid,
            tc.tile_pool(name="acc", bufs=1) as accp,
        ):
            accA = accp.tile([_P, T], f32)  # per-tile per-partition sum(WS2)
            absA = accp.tile([_P, T], f32)  # per-tile per-partition sum(|WS2|)
            neg1 = accp.tile([_P, 1], f32)  # bias vector for Sign activation
            nc.vector.memset(neg1, -1.0)
            for i in range(T):
                pt = io.tile([_P, 2 * F], f32, tag="probs")
                nc.sync.dma_start(out=pt, in_=probs[i])
                lt = io.tile([_P, labw * F], i32, tag="labs")
                nc.sync.dma_start(out=lt, in_=labs[i])
                ut = io.tile([_P, F], f32, tag="unc")
                nc.sync.dma_start(out=ut, in_=unc[i])

                p1 = pt[:, 1::2]  # confidences, strided view of interleaved probs
                ll = lt[:, 0::2] if lab_mode == "i64" else lt[:, :]

                tt = mid.tile([_P, F], bf16, tag="tt")
                nc.scalar.activation(tt, ut, Act.Tanh)
                c2 = mid.tile([_P, F], bf16, tag="c2")
                nc.scalar.activation(c2, p1, Act.Copy, bias=-1.0, scale=2.0)
                sg = mid.tile([_P, F], bf16, tag="sg")
                nc.scalar.activation(sg, p1, Act.Sign, bias=neg1, scale=2.0)
                l2 = mid.tile([_P, F], bf16, tag="l2")
                nc.vector.tensor_scalar(
                    out=l2, in0=ll, scalar1=2.0, scalar2=-1.0,
                    op0=Alu.mult, op1=Alu.add,
                )
                hm = mid.tile([_P, F], bf16, tag="hm")
                nc.vector.scalar_tensor_tensor(
                    hm, ut, float(unc_th), tt, op0=Alu.is_le, op1=Alu.subtract
                )
                sa = mid.tile([_P, F], bf16, tag="sa")
                nc.vector.tensor_mul(sa, l2, sg)
                g = mid.tile([_P, F], bf16, tag="g")
                nc.vector.tensor_add(g, sa, c2)
                ws = mid.tile([_P, F], bf16, tag="ws")
                nc.vector.scalar_tensor_tensor(
                    ws, g, 0.0, hm, op0=Alu.bypass, op1=Alu.mult,
                    accum_out=accA[:, i : i + 1],
                )
                nc.vector.tensor_reduce(
                    absA[:, i : i + 1], ws, axis=mybir.AxisListType.X,
                    op=Alu.add, apply_absolute_value=True,
                )
            nc.sync.dma_start(out=out[:, 0:T], in_=accA)
            nc.sync.dma_start(out=out[:, T : 2 * T], in_=absA)
    nc.finalize()  # Bacc: run wait-splitting + register allocation passes
    return nc


def _prep(probs, labels, unc, unc_th):
    probs = np.ascontiguousarray(np.asarray(probs), dtype=np.float32)
    unc = np.ascontiguousarray(np.asarray(unc), dtype=np.float32)
    labels = np.ascontiguousarray(np.asarray(labels))
    th = float(np.asarray(unc_th))
    assert probs.shape == (_N, 2), probs.shape
    assert unc.shape == (_N,), unc.shape
    assert labels.shape == (_N,), labels.shape

    if labels.dtype == np.int64:
        lab_mode = "i64"
        lab32 = labels.view(np.int32).reshape(_NCORES, _T, _P, 2 * _F)
    else:
        lab_mode = "i32"
        lab32 = labels.astype(np.int32, copy=False).reshape(_NCORES, _T, _P, _F)

    key = (th, lab_mode)
    if key not in _built:
        _built[key] = _build(th, lab_mode)
    nc = _built[key]

    pr = probs.reshape(_NCORES, _T, _P, 2 * _F)
    un = unc.reshape(_NCORES, _T, _P, _F)
    in_maps = [
        {"probs": pr[c], "labs": lab32[c], "unc": un[c]} for c in range(_NCORES)
    ]
    return nc, in_maps


def _finish(results):
    S_ws = 0.0
    S_abs = 0.0
    for r in results:
        o = r["out"].astype(np.float64)
        S_ws += o[:, :_T].sum()
        S_abs += o[:, _T:].sum()
    den = S_abs / 2.0
    num = (S_abs + S_ws) / 4.0
    avu = num / (den + 1e-10)
    loss = -1.0 * np.log(avu + 1e-10)
    return np.asarray([loss], dtype=np.float32)


def _run(probs, labels, unc, unc_th, trace=False, **kwargs):
    from concourse.bass_utils import run_bass_kernel_spmd

    nc, in_maps = _prep(probs, labels, unc, unc_th)
    res = run_bass_kernel_spmd(
        nc, in_maps, core_ids=list(range(_NCORES)), trace=trace, **kwargs
    )
    return _finish(res.results), res


def kernel(probs, labels, unc, unc_th):
    out, _ = _run(probs, labels, unc, unc_th, trace=False)
    return out


# revision 10
# speedup vs baseline: 1.0529x; 1.0529x over previous
"""AvU loss (accuracy-vs-uncertainty) Trainium2 kernel.

Strategy (data parallel over 8 NeuronCores):
  Each sample contributes w = q*r to the denominator and w*[a==u] to the
  numerator, where
     q = c if accurate else (1-c),        c = probs[:,1]
     r = (1-t) if certain else t,         t = tanh(unc)
     a = [label == argmax(probs)],        u = [unc <= unc_th]
  With sign encodings S_a = 2a-1, S_u = 2u-1 (both +-1):
     WS2 := (S_a + c2) * (u01 - t)  where c2 = 2c-1, u01 = [unc<=th]
          = 2 * w * S_a * S_u
  so   sum(w)        = sum(|WS2|) / 2
       sum(w*[a==u]) = (sum(|WS2|) + sum(WS2)) / 4
  Each core computes per-partition partial sums of WS2 and |WS2|; the host
  combines 8 * 128 * T partials in float64 and finishes the log.
"""

import numpy as np

_N = 16777216
_NCORES = 8
_P = 128
_F = 2048
_NC = _N // _NCORES
_T = _NC // (_P * _F)  # 8 tiles per core

_built = {}


def _build(unc_th: float, lab_mode: str, T=_T, F=_F):
    import concourse.bacc as bacc
    import concourse.mybir as mybir
    import concourse.tile as tile

    f32 = mybir.dt.float32
    bf16 = mybir.dt.bfloat16
    i32 = mybir.dt.int32
    Alu = mybir.AluOpType
    Act = mybir.ActivationFunctionType

    nc = bacc.Bacc("TRN2")
    labw = 2 if lab_mode == "i64" else 1
    probs = nc.dram_tensor("probs", [T, _P, 2 * F], f32, kind="ExternalInput")
    labs = nc.dram_tensor("labs", [T, _P, labw * F], i32, kind="ExternalInput")
    unc = nc.dram_tensor("unc", [T, _P, F], f32, kind="ExternalInput")
    out = nc.dram_tensor("out", [_P, T], f32, kind="ExternalOutput")
    out_abs = nc.dram_tensor("out_abs", [1, 512], f32, kind="ExternalOutput")

    CH = F // 512  # 512-col matmul chunks (one PSUM bank per matmul)
    with tile.TileContext(nc) as tc:
        with (
            tc.tile_pool(name="io", bufs=3 if lab_mode == "i32" else 2) as io,
            tc.tile_pool(name="mid", bufs=2) as mid,
            tc.tile_pool(name="acc", bufs=1) as accp,
            tc.tile_pool(name="ps", bufs=1, space="PSUM") as psp,
        ):
            accA = accp.tile([_P, T], f32)  # per-tile per-partition sum(WS2)
            neg1 = accp.tile([_P, 1], f32)  # bias vector for Sign activation
            nc.vector.memset(neg1, -1.0)
            ones = accp.tile([_P, 1], bf16)  # matmul ones-vector (column sums)
            nc.vector.memset(ones, 1.0)
            ps_abs = psp.tile([1, 512], f32)  # accumulates sum(|WS2|)
            for i in range(T):
                pt = io.tile([_P, 2 * F], f32, tag="probs")
                nc.sync.dma_start(out=pt, in_=probs[i])
                lt = io.tile([_P, labw * F], i32, tag="labs")
                nc.sync.dma_start(out=lt, in_=labs[i])
                ut = io.tile([_P, F], f32, tag="unc")
                nc.sync.dma_start(out=ut, in_=unc[i])

                p1 = pt[:, 1::2]  # confidences, strided view of interleaved probs
                ll = lt[:, 0::2] if lab_mode == "i64" else lt[:, :]

                tt = mid.tile([_P, F], bf16, tag="tt")
                nc.scalar.activation(tt, ut, Act.Tanh)
                c2 = mid.tile([_P, F], bf16, tag="c2")
                nc.scalar.activation(c2, p1, Act.Copy, bias=-1.0, scale=2.0)
                sg = mid.tile([_P, F], bf16, tag="sg")
                nc.scalar.activation(sg, p1, Act.Sign, bias=neg1, scale=2.0)
                l2 = mid.tile([_P, F], bf16, tag="l2")
                nc.vector.tensor_scalar(
                    out=l2, in0=ll, scalar1=2.0, scalar2=-1.0,
                    op0=Alu.mult, op1=Alu.add,
                )
                hm = mid.tile([_P, F], bf16, tag="hm")
                nc.vector.scalar_tensor_tensor(
                    hm, ut, float(unc_th), tt, op0=Alu.is_le, op1=Alu.subtract
                )
                sa = mid.tile([_P, F], bf16, tag="sa")
                nc.vector.tensor_mul(sa, l2, sg)
                g = mid.tile([_P, F], bf16, tag="g")
                nc.vector.tensor_add(g, sa, c2)
                ws = mid.tile([_P, F], bf16, tag="ws")
                nc.vector.scalar_tensor_tensor(
                    ws, g, 0.0, hm, op0=Alu.bypass, op1=Alu.mult,
                    accum_out=accA[:, i : i + 1],
                )
                aw = mid.tile([_P, F], bf16, tag="aw")
                nc.scalar.activation(aw, ws, Act.Abs)
                for c in range(CH):
                    nc.tensor.matmul(
                        ps_abs, ones, aw[:, c * 512 : (c + 1) * 512],
                        start=(i == 0 and c == 0),
                        stop=(i == T - 1 and c == CH - 1),
                    )
            stage = accp.tile([1, 512], f32)
            nc.scalar.copy(out=stage, in_=ps_abs)
            nc.sync.dma_start(out=out[:, :], in_=accA)
            nc.sync.dma_start(out=out_abs[:, :], in_=stage)
    nc.finalize()  # Bacc: run wait-splitting + register allocation passes
    return nc


def _prep(probs, labels, unc, unc_th):
    probs = np.ascontiguousarray(np.asarray(probs), dtype=np.float32)
    unc = np.ascontiguousarray(np.asarray(unc), dtype=np.float32)
    labels = np.ascontiguousarray(np.asarray(labels))
    th = float(np.asarray(unc_th))
    assert probs.shape == (_N, 2), probs.shape
    assert unc.shape == (_N,), unc.shape
    assert labels.shape == (_N,), labels.shape

    if labels.dtype == np.int64:
        lab_mode = "i64"
        lab32 = labels.view(np.int32).reshape(_NCORES, _T, _P, 2 * _F)
    else:
        lab_mode = "i32"
        lab32 = labels.astype(np.int32, copy=False).reshape(_NCORES, _T, _P, _F)

    key = (th, lab_mode)
    if key not in _built:
        _built[key] = _build(th, lab_mode)
    nc = _built[key]

    pr = probs.reshape(_NCORES, _T, _P, 2 * _F)
    un = unc.reshape(_NCORES, _T, _P, _F)
    in_maps = [
        {"probs": pr[c], "labs": lab32[c], "unc": un[c]} for c in range(_NCORES)
    ]
    return nc, in_maps


def _finish(results):
    S_ws = 0.0
    S_abs = 0.0
    for r in results:
        S_ws += r["out"].astype(np.float64).sum()
        S_abs += r["out_abs"].astype(np.float64).sum()
    den = S_abs / 2.0
    num = (S_abs + S_ws) / 4.0
    avu = num / (den + 1e-10)
    loss = -1.0 * np.log(avu + 1e-10)
    return np.asarray([loss], dtype=np.float32)


def _run(probs, labels, unc, unc_th, trace=False, **kwargs):
    from concourse.bass_utils import run_bass_kernel_spmd

    nc, in_maps = _prep(probs, labels, unc, unc_th)
    res = run_bass_kernel_spmd(
        nc, in_maps, core_ids=list(range(_NCORES)), trace=trace, **kwargs
    )
    return _finish(res.results), res


def kernel(probs, labels, unc, unc_th):
    out, _ = _run(probs, labels, unc, unc_th, trace=False)
    return out
